# revision 3
# baseline (speedup 1.0000x reference)
"""Hashed-weight MLP (1024-4096-4096-32000, batch 2048) on 8 TRN2 NeuronCores.

Problem: h = relu(x @ W0); h = relu(h @ W1); out = h @ W2, where each
W_l[i, j] = hw_l[(a_l*i + b_l*j + c_l) % N_l] is a virtual (ROBE-Z hashed)
weight gathered from a small parameter vector.

v2 approach (column-parallel tensor parallelism on all three layers):
  * Through the host-permuted table hw_bb[t] = hw[(b*t) % N], the virtual
    weight is ROW-CONTIGUOUS: W[i, j0+dj] = hw_bb[u_i + j0 + dj] with row
    starts u_i in arithmetic progression (stride q = b^-1 a mod N).
  * L0/L1 use a ladder k chosen with POSITIVE residue r = q*k mod N and the
    contraction order permuted to ladder-lex (i = i1 + k*i2, i1-major): each
    [128, w] weight tile is then a handful of (stride r, contiguous w) reads
    straight from the hash slice INTO SBUF - no DRAM staging. The rhs
    activations are gathered with the matching row stride k.
  * L2's residue is negative (r=-943), which the BIR verifier forbids as an
    SBUF-destination outer dim, so L2 stages to DRAM via the classic ladder
    (natural row order) and loads contiguous slabs - but staging is issued
    at t~0 on the scalar ring, interleaved with slab loads per j-group.
  * ReLUs and PSUM->SBUF copies run on the vector engine so the scalar/sync
    DMA queues never block PSUM drain (the v1 156us stall).
  * Output is written bf16 (error budget allows) and upcast on host.
"""
import sys
if "/opt/trn_rl_repo" not in sys.path:
    sys.path.insert(0, "/opt/trn_rl_repo")

import numpy as np
import ml_dtypes

import concourse.bass as bass
import concourse.bacc as bacc
import concourse.tile as tile
import concourse.mybir as mybir
from concourse.bass_utils import run_bass_kernel_spmd

N_CORES = 8
P = 128
NB = 512                      # batch tile (PSUM bank limit)
BATCH = 2048
BT = BATCH // NB              # 4

LENS = [1024, 4096, 4096, 32000]
HASH_A = [9973, 10007, 10039]
HASH_B = [31013, 31019, 31039]
HASH_C = [557, 563, 569]
SIZES = [1048576, 1048576, 4194304]

JW = [512, 512, 4000]         # true per-core output shard width
WMAT = [512, 512, 4096]       # materialized width (L2 padded to 32 j-tiles)

BF = mybir.dt.bfloat16
F32 = mybir.dt.float32


def _plan_layer(l, positive_r):
    N = SIZES[l]; a, b, ch = HASH_A[l], HASH_B[l], HASH_C[l]
    binv = pow(b, -1, N)
    q = (binv * a) % N
    u0 = (binv * ch) % N
    in_dim = LENS[l]; w = WMAT[l]
    best = None
    for k in range(1, min(in_dim, 600) + 1):
        r = (q * k) % N
        if r > N // 2:
            r -= N
        if positive_r and r <= 0:
            continue
        C1 = -(-in_dim // k)
        extra = q * (k - 1) + abs(r) * (C1 - 1)
        if best is None or extra < best[0]:
            best = (extra, k, C1, r)
    _, k, C1, r = best
    shift = max(0, -r * (C1 - 1))
    m_ext = shift + q * (k - 1) + max(r, 0) * (C1 - 1) + w + 64
    return dict(N=N, a=a, b=b, ch=ch, q=q, u0=u0, k=k, C1=C1, r=r,
                shift=shift, m_ext=m_ext, rows=k * C1, in_dim=in_dim, w=w)


# L0/L1: positive-r ladders (direct-to-SBUF); L2: classic (stages via DRAM)
PLANS = [_plan_layer(0, True), _plan_layer(1, True), _plan_layer(2, False)]
RG = [list(range(N_CORES))]


def _tile_segments(pl):
    """Ladder-lex tiling of [0, in_dim): position p -> (i1, i2) i1-major.
    Returns per-128-tile segment lists (i1, i2_start, length, p_off)."""
    k, in_dim = pl["k"], pl["in_dim"]
    segs = []
    for i1 in range(k):
        n = (in_dim - 1 - i1) // k + 1
        segs.append((i1, 0, n))
    ntile = in_dim // P
    tiles = [[] for _ in range(ntile)]
    p = 0
    for (i1, i2s, n) in segs:
        while n > 0:
            t, off = divmod(p, P)
            take = min(n, P - off)
            tiles[t].append((i1, i2s, take, off))
            p += take; i2s += take; n -= take
    assert p == in_dim
    return tiles


TILES0 = _tile_segments(PLANS[0])
TILES1 = _tile_segments(PLANS[1])


def build_nc():
    nc = bacc.Bacc("TRN2", target_bir_lowering=False, debug=False,
                   num_devices=N_CORES)

    xT_d = nc.dram_tensor("xT", [LENS[0], BATCH], BF, kind="ExternalInput").ap()
    hb = [nc.dram_tensor(f"hb{l}", [PLANS[l]["m_ext"]], BF,
                         kind="ExternalInput").ap() for l in range(3)]
    # L2 staging, one tensor per j-group for per-slab dep tracking
    w2_jg = [nc.dram_tensor(f"w2jg{g}", [PLANS[2]["rows"], 1024], BF).ap()
             for g in range(4)]
    h1c = [nc.dram_tensor(f"h1c{b}", [512, NB], BF).ap() for b in range(BT)]
    h1f = [nc.dram_tensor(f"h1f{b}", [4096, NB], BF, addr_space="Shared").ap()
           for b in range(BT)]
    h2c = [nc.dram_tensor(f"h2c{b}", [512, NB], BF).ap() for b in range(BT)]
    h2f = [nc.dram_tensor(f"h2f{b}", [4096, NB], BF, addr_space="Shared").ap()
           for b in range(BT)]
    out_d = nc.dram_tensor("outT", [4096, BATCH], BF, kind="ExternalOutput").ap()

    def load_lex_tile(eng, dst_tile, pl, segs, hbl, w, jg_off=0):
        """Weight tile [128, w] straight from the hash slice (r > 0)."""
        q, r, shift = pl["q"], pl["r"], pl["shift"]
        for (i1, i2s, ln, off) in segs:
            base = shift + q * i1 + r * i2s + jg_off
            eng.dma_start(
                out=bass.AP(dst_tile.tensor, off * w, [[w, ln], [1, w]]),
                in_=bass.AP(hbl.tensor, base, [[r, ln], [1, w]]))

    def load_rhs_tile(eng, dst_tile, pl, segs, src, src_w):
        """Activation rows i = i1 + k*i2 gathered at row stride k."""
        k = pl["k"]
        for (i1, i2s, ln, off) in segs:
            row = i1 + k * i2s
            eng.dma_start(
                out=bass.AP(dst_tile.tensor, off * src_w,
                            [[src_w, ln], [1, src_w]]),
                in_=bass.AP(src.tensor, row * src_w,
                            [[k * src_w, ln], [1, src_w]]))

    def matz2(g, eng):
        """Stage L2 j-group g: hash slice -> natural-row DRAM [rows, 1024]."""
        pl = PLANS[2]
        q, k, C1, r = pl["q"], pl["k"], pl["C1"], pl["r"]
        w = 1024
        nchunk = 4
        step = -(-C1 // nchunk)
        for ci in range(nchunk):
            c1a = ci * step
            cnt = min(C1, c1a + step) - c1a
            src = bass.AP(hb[2].tensor, pl["shift"] + g * 1024 + r * c1a,
                          [[q, k], [r, cnt], [1, w]])
            dst = bass.AP(w2_jg[g].tensor, k * w * c1a,
                          [[w, k], [k * w, cnt], [1, w]])
            eng.dma_start(out=dst, in_=src)

    with tile.TileContext(nc) as tc, \
         tc.tile_pool(name="ps", bufs=8, space="PSUM") as psp, \
         tc.tile_pool(name="slabA", bufs=1) as slabA, \
         nc.allow_non_contiguous_dma(reason="hash ladder tiles"):

        # ---------- startup: W0+x tiles interleaved, then W1, then L2 staging
        with tc.tile_pool(name="l1w", bufs=1) as l1wp, \
             tc.tile_pool(name="l1r", bufs=6) as l1rp, \
             tc.tile_pool(name="l1o", bufs=8) as l1op:
            with tc.tile_pool(name="l0", bufs=1) as l0p:
                xsb = [l0p.tile([P, BATCH], BF, name=f"xsb{t}")
                       for t in range(8)]
                w0sb = [l0p.tile([P, 512], BF, name=f"w0sb{t}")
                        for t in range(8)]
                h1sb = [l0p.tile([P, BATCH], BF, name=f"h1sb{j}")
                        for j in range(4)]
                for t in range(8):
                    load_lex_tile(nc.scalar, w0sb[t], PLANS[0], TILES0[t],
                                  hb[0], 512)
                    load_rhs_tile(nc.sync, xsb[t], PLANS[0], TILES0[t],
                                  xT_d, BATCH)

                w1sb = [l1wp.tile([P, 512], BF, name=f"w1sb{t}")
                        for t in range(32)]
                for t in range(32):
                    load_lex_tile(nc.scalar, w1sb[t], PLANS[1], TILES1[t],
                                  hb[1], 512)

                matz2(0, nc.scalar)
                slab0 = [slabA.tile([P, 1024], BF, tag=f"w2slab{t}",
                                    name=f"w2s_0_{t}") for t in range(32)]
                for t in range(32):
                    nc.scalar.dma_start(out=slab0[t][:],
                                        in_=w2_jg[0][t * P:(t + 1) * P, :])
                matz2(1, nc.scalar)

                # ---------- Layer 0
                for b in range(BT):
                    for j in range(4):
                        ps = psp.tile([P, NB], F32, tag="ps",
                                      name=f"ps0_{b}_{j}")
                        for kt in range(8):
                            nc.tensor.matmul(
                                out=ps[:],
                                lhsT=w0sb[kt][:, j * P:(j + 1) * P],
                                rhs=xsb[kt][:, b * NB:(b + 1) * NB],
                                start=(kt == 0), stop=(kt == 7))
                        nc.vector.tensor_scalar_max(
                            out=h1sb[j][:, b * NB:(b + 1) * NB],
                            in0=ps[:], scalar1=0.0)
                    for j in range(4):
                        nc.sync.dma_start(
                            out=h1c[b][j * P:(j + 1) * P, :],
                            in_=h1sb[j][:, b * NB:(b + 1) * NB])
                    nc.gpsimd.collective_compute(
                        "AllGather", mybir.AluOpType.bypass,
                        replica_groups=RG,
                        ins=[h1c[b].opt()], outs=[h1f[b].opt()])

            # ---------- Layer 1 (l0 pools closed)
            for b in range(BT):
                    pss = [psp.tile([P, NB], F32, tag="ps",
                                    name=f"ps1_{b}_{j}") for j in range(4)]
                    for kt in range(32):
                        rhs = l1rp.tile([P, NB], BF, tag="l1rhs",
                                        name=f"l1r_{b}_{kt}")
                        load_rhs_tile(nc.sync, rhs, PLANS[1], TILES1[kt],
                                      h1f[b], NB)
                        for j in range(4):
                            nc.tensor.matmul(
                                out=pss[j][:],
                                lhsT=w1sb[kt][:, j * P:(j + 1) * P],
                                rhs=rhs[:],
                                start=(kt == 0), stop=(kt == 31))
                    for j in range(4):
                        h2o = l1op.tile([P, NB], BF, tag="h2o",
                                        name=f"h2o_{b}_{j}")
                        nc.vector.tensor_scalar_max(out=h2o[:], in0=pss[j][:],
                                                    scalar1=0.0)
                        nc.sync.dma_start(out=h2c[b][j * P:(j + 1) * P, :],
                                          in_=h2o[:])
                    nc.gpsimd.collective_compute(
                        "AllGather", mybir.AluOpType.bypass,
                        replica_groups=RG,
                        ins=[h2c[b].opt()], outs=[h2f[b].opt()])

        # ---------- Layer 2 (slabbed by j-group, h2f streamed)
        with tc.tile_pool(name="slabB", bufs=1) as slabB, \
             tc.tile_pool(name="l2r", bufs=6) as l2rp, \
             tc.tile_pool(name="l2o", bufs=16) as l2op:
            for jg in range(4):
                if jg == 0:
                    slab = slab0
                else:
                    pool = slabA if jg % 2 == 0 else slabB
                    slab = [pool.tile([P, 1024], BF, tag=f"w2slab{t}",
                                      name=f"w2s_{jg}_{t}") for t in range(32)]
                    for t in range(32):
                        nc.scalar.dma_start(
                            out=slab[t][:],
                            in_=w2_jg[jg][t * P:(t + 1) * P, :])
                if jg + 2 < 4:
                    matz2(jg + 2, nc.scalar)
                for b in range(BT):
                    pss = [psp.tile([P, NB], F32, tag="ps",
                                    name=f"ps2_{jg}_{b}_{j}")
                           for j in range(8)]
                    for kt in range(32):
                        rhs = l2rp.tile([P, NB], BF, tag="l2rhs",
                                        name=f"l2r_{jg}_{b}_{kt}")
                        nc.sync.dma_start(out=rhs[:],
                                          in_=h2f[b][kt * P:(kt + 1) * P, :])
                        for j in range(8):
                            nc.tensor.matmul(
                                out=pss[j][:],
                                lhsT=slab[kt][:, j * P:(j + 1) * P],
                                rhs=rhs[:],
                                start=(kt == 0), stop=(kt == 31))
                    for j in range(8):
                        osb = l2op.tile([P, NB], BF, tag="l2out",
                                        name=f"l2o_{jg}_{b}_{j}")
                        nc.vector.tensor_copy(out=osb[:], in_=pss[j][:])
                        nc.scalar.dma_start(
                            out=out_d[(jg * 8 + j) * P:(jg * 8 + j + 1) * P,
                                      b * NB:(b + 1) * NB],
                            in_=osb[:])

    nc.compile()
    return nc


_NC_CACHE = None


def _get_nc():
    global _NC_CACHE
    if _NC_CACHE is None:
        _NC_CACHE = build_nc()
    return _NC_CACHE


def _prep_inputs(x, hw0, hw1, hw2):
    """Host prep: transpose x, build per-core periodic permuted-table slices."""
    x = np.asarray(x, np.float32)
    hws = [np.asarray(hw0, np.float32), np.asarray(hw1, np.float32),
           np.asarray(hw2, np.float32)]
    xT = np.ascontiguousarray(x.T).astype(ml_dtypes.bfloat16)

    per_core_hb = [[None] * 3 for _ in range(N_CORES)]
    for l in range(3):
        pl = PLANS[l]
        N, b = pl["N"], pl["b"]
        m_ext = pl["m_ext"]
        jw = JW[l]
        t0 = pl["u0"] - pl["shift"]          # core-0 slice start (in t-space)
        span = m_ext + (N_CORES - 1) * jw
        t = t0 + np.arange(span, dtype=np.int64)
        shared = hws[l][(b * t) % N].astype(ml_dtypes.bfloat16)
        for c in range(N_CORES):
            per_core_hb[c][l] = shared[c * jw: c * jw + m_ext]
    in_maps = []
    for c in range(N_CORES):
        in_maps.append({
            "xT": xT,
            "hb0": per_core_hb[c][0],
            "hb1": per_core_hb[c][1],
            "hb2": per_core_hb[c][2],
        })
    return in_maps


def kernel(x, hw0, hw1, hw2, trace=False):
    nc = _get_nc()
    in_maps = _prep_inputs(x, hw0, hw1, hw2)
    res = run_bass_kernel_spmd(nc, in_maps, list(range(N_CORES)), trace=trace)
    outs = [res.results[c]["outT"][:JW[2], :] for c in range(N_CORES)]
    full = np.concatenate(outs, axis=0)         # [32000, 2048] bf16
    out = np.ascontiguousarray(full.T).astype(np.float32)
    kernel.last_results = res
    return out
